# revision 16
# baseline (speedup 1.0000x reference)
"""Trainium2 Bass kernel for nn_FPModule (gnn message passing / FPModule).

Sharding: 2 graphs per core x 8 cores. Per core:
  - KNN scores u = 2*y.x - ||x||^2 on PE via 21-row 3-way-bf16-split matmul
    (error ~2e-7 rel, fp32-accurate ranking).
  - top-3 via DVE max8 + max_index (exact fp32).
  - interp weights from score values: d_k = ysq - v_k (clamped), w=1/d.
  - z = Rx^T x per coarse node (fp16) staged in DRAM; batched indirect-DMA
    gather of 12 rows/partition; interp on ACT (scale-AP), rotation by Ry via
    broadcast-AP DVE ops; fp16 MLP on PE; output written feature-major,
    host transposes back.
"""
import os
import numpy as np
from contextlib import ExitStack

import ml_dtypes
import concourse.bass as bass
import concourse.bacc as bacc
import concourse.mybir as mybir
from concourse.tile import TileContext
from concourse.bass import IndirectOffsetOnAxis
from concourse.masks import make_identity

DT = mybir.dt
AL = mybir.AluOpType
AF = mybir.ActivationFunctionType

NCORE = 8
B = 16
GPC = 2               # graphs per core
NP = 1024             # coarse per graph
MP = 4096             # queries per graph
NL = GPC * NP         # 2048 coarse rows per core
ML = GPC * MP         # 8192 query rows per core
NVEC = 64
FD = 192
FSK = 64
HID = 256
OUT = 128
QT = 128              # queries per tile
NT = MP // QT         # 32 tiles per graph
BT = 4                # tile batch

_CACHE = {}


def _build_nc():
    nc = bacc.Bacc("TRN2", target_bir_lowering=False, debug=False)
    dt = nc.dram_tensor
    x_in = dt("x_in", [NL, FD], DT.float32, kind="ExternalInput")
    lf_in = dt("lf_in", [NL, 9], DT.float32, kind="ExternalInput")
    lfs_in = dt("lfs_in", [ML, 9], DT.float32, kind="ExternalInput")
    pos_in = dt("pos_in", [NL, 3], DT.float32, kind="ExternalInput")
    psk_in = dt("psk_in", [ML, 3], DT.float32, kind="ExternalInput")
    xh_in = dt("xh_in", [GPC, 3, NP], DT.bfloat16, kind="ExternalInput")
    xm_in = dt("xm_in", [GPC, 3, NP], DT.bfloat16, kind="ExternalInput")
    xl_in = dt("xl_in", [GPC, 3, NP], DT.bfloat16, kind="ExternalInput")
    x2n_in = dt("x2n_in", [GPC, 3, NP], DT.bfloat16, kind="ExternalInput")
    yh_in = dt("yh_in", [GPC, 3, MP], DT.bfloat16, kind="ExternalInput")
    ym_in = dt("ym_in", [GPC, 3, MP], DT.bfloat16, kind="ExternalInput")
    yl_in = dt("yl_in", [GPC, 3, MP], DT.bfloat16, kind="ExternalInput")
    xskT_in = dt("xskT_in", [FSK, ML], DT.float16, kind="ExternalInput")
    w1_in = dt("w1_in", [FD + FSK, HID], DT.float16, kind="ExternalInput")
    b1_in = dt("b1_in", [HID, 1], DT.float32, kind="ExternalInput")
    w2_in = dt("w2_in", [HID, OUT], DT.float16, kind="ExternalInput")
    b2_in = dt("b2_in", [1, OUT], DT.float16, kind="ExternalInput")
    ones3_in = dt("ones3_in", [3, QT], DT.bfloat16, kind="ExternalInput")
    zs = [dt("z_%d" % g, [NP, FD], DT.float16, kind="Internal") for g in range(GPC)]
    outT = dt("outT", [OUT, ML], DT.float32, kind="ExternalOutput")

    with TileContext(nc) as tc, ExitStack() as ctx:
        const = ctx.enter_context(tc.tile_pool(name="const", bufs=1))
        xpool = ctx.enter_context(tc.tile_pool(name="xg", bufs=2))
        zpool = ctx.enter_context(tc.tile_pool(name="zp", bufs=2))
        ypool = ctx.enter_context(tc.tile_pool(name="yp", bufs=6))
        upool = ctx.enter_context(tc.tile_pool(name="up", bufs=2, space="PSUM"))
        usbp = ctx.enter_context(tc.tile_pool(name="usb", bufs=2))
        smpool = ctx.enter_context(tc.tile_pool(name="sm", bufs=2))
        gpool = ctx.enter_context(tc.tile_pool(name="gp", bufs=2))
        spool = ctx.enter_context(tc.tile_pool(name="sp", bufs=2))
        hpool = ctx.enter_context(tc.tile_pool(name="hp", bufs=2))
        mpsum = ctx.enter_context(tc.tile_pool(name="mp", bufs=1, space="PSUM"))
        tpsum = ctx.enter_context(tc.tile_pool(name="tp", bufs=2, space="PSUM"))

        # ---- constants ----
        w1t = const.tile([128, 2, HID], DT.float16)
        nc.sync.dma_start(out=w1t[:, 0, :], in_=w1_in[0:128, :])
        nc.sync.dma_start(out=w1t[:, 1, :], in_=w1_in[128:256, :])
        w2t = const.tile([128, 2, OUT], DT.float16)
        nc.sync.dma_start(out=w2t[:, 0, :], in_=w2_in[0:128, :])
        nc.sync.dma_start(out=w2t[:, 1, :], in_=w2_in[128:256, :])
        b1t = const.tile([128, 2], DT.float32)
        nc.sync.dma_start(out=b1t[:, 0:1], in_=b1_in[0:128, :])
        nc.sync.dma_start(out=b1t[:, 1:2], in_=b1_in[128:256, :])
        b2t = const.tile([1, OUT], DT.float16)
        nc.sync.dma_start(out=b2t[:], in_=b2_in[:])
        onesrow = const.tile([1, BT * QT], DT.float16)
        nc.vector.memset(onesrow[:], 1.0)
        ident = const.tile([128, 128], DT.float16)
        make_identity(nc, ident[:])

        for g in range(GPC):
            # ---- X moving tile [21, NP] ----
            Xg = xpool.tile([21, NP], DT.bfloat16)
            nc.sync.dma_start(out=Xg[0:3, :], in_=xh_in[g])
            nc.sync.dma_start(out=Xg[3:6, :], in_=xm_in[g])
            nc.sync.dma_start(out=Xg[6:9, :], in_=xh_in[g])
            nc.sync.dma_start(out=Xg[9:12, :], in_=xl_in[g])
            nc.sync.dma_start(out=Xg[12:15, :], in_=xh_in[g])
            nc.sync.dma_start(out=Xg[15:18, :], in_=xm_in[g])
            # x2 rows: negated ||x||^2 split into 3 bf16 (host-computed)
            nc.sync.dma_start(out=Xg[18:21, :], in_=x2n_in[g])

            # ---- ysq for this graph: [128, NT] ----
            yrm = zpool.tile([128, NT, 3], DT.float32)
            nc.sync.dma_start(
                out=yrm[:],
                in_=psk_in[g * MP:(g + 1) * MP, :].rearrange("(t p) d -> p t d", p=128))
            ysq = smpool.tile([128, NT], DT.float32, tag="ysq")
            sq2 = zpool.tile([128, NT, 3], DT.float32)
            nc.vector.tensor_tensor(sq2[:], yrm[:], yrm[:], op=AL.mult)
            nc.vector.tensor_reduce(ysq[:], sq2[:], axis=mybir.AxisListType.X, op=AL.add)

            # ---- z phase: z = Rx^T x, fp16, layout z[c, 3v+a] ----
            for t0 in range(0, NP // QT, BT):
                xt = zpool.tile([128, BT, FD], DT.float32, tag="zx")
                nc.sync.dma_start(
                    out=xt[:],
                    in_=x_in[g * NP + t0 * 128:g * NP + (t0 + BT) * 128, :]
                    .rearrange("(t p) f -> p t f", p=128))
                lft = zpool.tile([128, BT, 9], DT.float32, tag="zlf")
                nc.sync.dma_start(
                    out=lft[:],
                    in_=lf_in[g * NP + t0 * 128:g * NP + (t0 + BT) * 128, :]
                    .rearrange("(t p) f -> p t f", p=128))
                pb = []
                for b in range(3):
                    pbt = zpool.tile([128, BT, FD], DT.float16, tag="zpb%d" % b,
                                     name="pb%d" % b)
                    pb.append(pbt)
                for b in range(3):
                    for t in range(BT):
                        for a in range(3):
                            # p_b[:, t, 3v+a] = x[:, t, 3v+b] * Rx[b, a]
                            nc.scalar.activation(
                                pb[b][:, t, a:FD:3], xt[:, t, b:FD:3], AF.Copy,
                                bias=0.0, scale=lft[:, t, 3 * b + a:3 * b + a + 1])
                zt = zpool.tile([128, BT, FD], DT.float16, tag="zt")
                nc.vector.tensor_tensor(zt[:], pb[0][:], pb[1][:], op=AL.add)
                nc.vector.tensor_tensor(zt[:], zt[:], pb[2][:], op=AL.add)
                nc.sync.dma_start(
                    out=zs[g][t0 * 128:(t0 + BT) * 128, :]
                    .rearrange("(t p) f -> p t f", p=128),
                    in_=zt[:])

            # ---- query tiles ----
            for jb in range(0, NT, BT):
                vb = smpool.tile([128, BT, 8], DT.float32, tag="vb")
                ib = smpool.tile([128, BT, 8], DT.uint32, tag="ib")
                for jj in range(BT):
                    j = jb + jj
                    Y = ypool.tile([21, QT], DT.bfloat16)
                    c0 = j * QT
                    nc.sync.dma_start(out=Y[0:3, :], in_=yh_in[g, :, c0:c0 + QT])
                    nc.sync.dma_start(out=Y[3:6, :], in_=yh_in[g, :, c0:c0 + QT])
                    nc.sync.dma_start(out=Y[6:9, :], in_=ym_in[g, :, c0:c0 + QT])
                    nc.sync.dma_start(out=Y[9:12, :], in_=yh_in[g, :, c0:c0 + QT])
                    nc.sync.dma_start(out=Y[12:15, :], in_=yl_in[g, :, c0:c0 + QT])
                    nc.sync.dma_start(out=Y[15:18, :], in_=ym_in[g, :, c0:c0 + QT])
                    nc.sync.dma_start(out=Y[18:21, :], in_=ones3_in[:])
                    up = upool.tile([128, NP], DT.float32)
                    nc.tensor.matmul(up[:, 0:512], lhsT=Y[:], rhs=Xg[:, 0:512],
                                     start=True, stop=True)
                    nc.tensor.matmul(up[:, 512:1024], lhsT=Y[:], rhs=Xg[:, 512:1024],
                                     start=True, stop=True)
                    usb = usbp.tile([128, NP], DT.float32)
                    nc.scalar.activation(usb[:], up[:], AF.Copy, bias=0.0, scale=1.0)
                    nc.vector.max(vb[:, jj, :], usb[:])
                    nc.vector.max_index(ib[:, jj, :], vb[:, jj, :], usb[:])

                # batched small ops for BT tiles
                tneg = smpool.tile([128, BT, 3], DT.float32, tag="tneg")
                nc.vector.tensor_tensor(
                    tneg[:], vb[:, :, 0:3],
                    ysq[:, jb:jb + BT, None].to_broadcast([128, BT, 3]),
                    op=AL.subtract)
                nc.vector.tensor_scalar_min(tneg[:], tneg[:], -1e-16)
                wn = smpool.tile([128, BT, 3], DT.float32, tag="wn")
                with nc.allow_low_precision(reason="1/d weights"):
                    nc.vector.reciprocal(wn[:], tneg[:])
                swn = smpool.tile([128, BT], DT.float32, tag="swn")
                nc.vector.tensor_reduce(swn[:], wn[:], axis=mybir.AxisListType.X,
                                        op=AL.add)
                rsw = smpool.tile([128, BT], DT.float32, tag="rsw")
                with nc.allow_low_precision(reason="1/sum"):
                    nc.vector.reciprocal(rsw[:], swn[:])
                wt = smpool.tile([128, BT, 3], DT.float32, tag="wt")
                nc.vector.tensor_tensor(
                    wt[:], wn[:], rsw[:, :, None].to_broadcast([128, BT, 3]),
                    op=AL.mult)
                ig = smpool.tile([128, BT, 3], DT.uint32, tag="ig")
                nc.vector.tensor_scalar(ig[:], ib[:, :, 0:3], 0, None, op0=AL.add)

                # gather 12 z rows per partition
                gb = gpool.tile([128, BT * 3, FD], DT.float16)
                for ii in range(BT * 3):
                    nc.gpsimd.indirect_dma_start(
                        out=gb[:, ii, :], out_offset=None, in_=zs[g][:],
                        in_offset=IndirectOffsetOnAxis(
                            ap=ig[:, ii // 3, ii % 3:ii % 3 + 1], axis=0))

                # interp on ACT: s = sum_k w_k * g_k
                s16 = spool.tile([128, BT, FD], DT.float16, tag="s16")
                t16 = spool.tile([128, BT, FD], DT.float16, tag="t16")
                for jj in range(BT):
                    nc.scalar.activation(s16[:, jj, :], gb[:, 3 * jj + 0, :], AF.Copy,
                                         bias=0.0, scale=wt[:, jj, 0:1])
                    nc.scalar.activation(t16[:, jj, :], gb[:, 3 * jj + 1, :], AF.Copy,
                                         bias=0.0, scale=wt[:, jj, 1:2])
                nc.vector.tensor_tensor(s16[:], s16[:], t16[:], op=AL.add)
                for jj in range(BT):
                    nc.scalar.activation(t16[:, jj, :], gb[:, 3 * jj + 2, :], AF.Copy,
                                         bias=0.0, scale=wt[:, jj, 2:3])
                nc.vector.tensor_tensor(s16[:], s16[:], t16[:], op=AL.add)

                # rotation y[q, a*64+v] = sum_b Ry[3a+b] * s[q, 3v+b]  (DVE bcast)
                ry = smpool.tile([128, BT, 9], DT.float32, tag="ry")
                for jj in range(BT):
                    q0 = g * MP + (jb + jj) * QT
                    nc.sync.dma_start(
                        out=ry[:, jj, :],
                        in_=lfs_in[q0:q0 + QT, :])
                y16 = hpool.tile([128, BT, 3, NVEC], DT.float16, tag="y16")
                u16 = hpool.tile([128, BT, 3, NVEC], DT.float16, tag="u16")

                def ry_view(b):
                    return ry[:, :, b:9:3, None].to_broadcast([128, BT, 3, NVEC])

                def s_view(b):
                    return s16[:, :, None, b:FD:3].to_broadcast([128, BT, 3, NVEC])

                nc.vector.tensor_tensor(y16[:], ry_view(0), s_view(0), op=AL.mult)
                nc.vector.tensor_tensor(u16[:], ry_view(1), s_view(1), op=AL.mult)
                nc.vector.tensor_tensor(y16[:], y16[:], u16[:], op=AL.add)
                nc.vector.tensor_tensor(u16[:], ry_view(2), s_view(2), op=AL.mult)
                nc.vector.tensor_tensor(y16[:], y16[:], u16[:], op=AL.add)

                # transposes to feature-major h
                hA = hpool.tile([128, BT, QT], DT.float16, tag="hA")
                hB = hpool.tile([128, BT, QT], DT.float16, tag="hB")
                yflat = y16[:].rearrange("p t a v -> p t (a v)")
                for jj in range(BT):
                    tp1 = tpsum.tile([128, QT], DT.float16, tag="tp")
                    nc.tensor.transpose(tp1[:], yflat[:, jj, 0:128], ident[:])
                    nc.scalar.activation(hA[:, jj, :], tp1[:], AF.Copy,
                                         bias=0.0, scale=1.0)
                    tp2 = tpsum.tile([64, QT], DT.float16, tag="tp")
                    nc.tensor.transpose(tp2[:], yflat[:, jj, 128:192], ident[:])
                    nc.scalar.activation(hB[0:64, jj, :], tp2[:], AF.Copy,
                                         bias=0.0, scale=1.0)
                    q0 = g * MP + (jb + jj) * QT
                    nc.sync.dma_start(out=hB[64:128, jj, :],
                                      in_=xskT_in[:, q0:q0 + QT])

                # MLP
                ps1a = mpsum.tile([128, BT * QT], DT.float32, tag="ps1")
                ps1b = mpsum.tile([128, BT * QT], DT.float32, tag="ps1")
                hAf = hA[:].rearrange("p t q -> p (t q)")
                hBf = hB[:].rearrange("p t q -> p (t q)")
                nc.tensor.matmul(ps1a[:], lhsT=w1t[:, 0, 0:128], rhs=hAf,
                                 start=True, stop=False)
                nc.tensor.matmul(ps1a[:], lhsT=w1t[:, 1, 0:128], rhs=hBf,
                                 start=False, stop=True)
                nc.tensor.matmul(ps1b[:], lhsT=w1t[:, 0, 128:256], rhs=hAf,
                                 start=True, stop=False)
                nc.tensor.matmul(ps1b[:], lhsT=w1t[:, 1, 128:256], rhs=hBf,
                                 start=False, stop=True)
                h2a = hpool.tile([128, BT * QT], DT.float16, tag="h2a")
                h2b = hpool.tile([128, BT * QT], DT.float16, tag="h2b")
                nc.scalar.activation(h2a[:], ps1a[:], AF.Relu,
                                     bias=b1t[:, 0:1], scale=1.0)
                nc.scalar.activation(h2b[:], ps1b[:], AF.Relu,
                                     bias=b1t[:, 1:2], scale=1.0)
                ps2 = mpsum.tile([128, BT * QT], DT.float32, tag="ps2")
                nc.tensor.matmul(ps2[:], lhsT=w2t[:, 0, :], rhs=h2a[:],
                                 start=True, stop=False)
                nc.tensor.matmul(ps2[:], lhsT=w2t[:, 1, :], rhs=h2b[:],
                                 start=False, stop=False)
                nc.tensor.matmul(ps2[:], lhsT=b2t[:], rhs=onesrow[:],
                                 start=False, stop=True)
                osb = hpool.tile([128, BT * QT], DT.float32, tag="osb")
                nc.scalar.activation(osb[:], ps2[:], AF.Copy, bias=0.0, scale=1.0)
                col0 = g * MP + jb * QT
                nc.sync.dma_start(out=outT[:, col0:col0 + BT * QT], in_=osb[:])
    return nc


def _split3(a):
    """3-way bf16 split of fp32 array: a ~= h + m + l (each bf16)."""
    h = a.astype(ml_dtypes.bfloat16)
    r = (a - h.astype(np.float32)).astype(np.float32)
    m = r.astype(ml_dtypes.bfloat16)
    l = (r - m.astype(np.float32)).astype(ml_dtypes.bfloat16)
    return h, m, l


def kernel(x, pos, pos_skip, x_skip, lframes, lframes_skip,
           W1, b1, W2, b2, batch, batch_skip):
    from concourse.bass_utils import run_bass_kernel_spmd

    x = np.asarray(x, dtype=np.float32)
    pos = np.asarray(pos, dtype=np.float32)
    pos_skip = np.asarray(pos_skip, dtype=np.float32)
    x_skip = np.asarray(x_skip, dtype=np.float32)
    lframes = np.asarray(lframes, dtype=np.float32)
    lframes_skip = np.asarray(lframes_skip, dtype=np.float32)
    W1 = np.asarray(W1, dtype=np.float32)
    b1 = np.asarray(b1, dtype=np.float32)
    W2 = np.asarray(W2, dtype=np.float32)
    b2 = np.asarray(b2, dtype=np.float32)

    if "nc" not in _CACHE:
        nc_ = _build_nc()
        nc_.finalize()
        _CACHE["nc"] = nc_
    nc = _CACHE["nc"]

    # W1 row permutation: y features (3v+a) -> (a*64+v); x_skip rows unchanged
    perm = np.empty(FD + FSK, dtype=np.int64)
    for v in range(NVEC):
        for a in range(3):
            perm[a * NVEC + v] = 3 * v + a
    perm[FD:] = np.arange(FD, FD + FSK)
    W1p = W1[perm, :].astype(np.float16)
    W2h = W2.astype(np.float16)
    b1c = b1.reshape(HID, 1).astype(np.float32)
    b2c = b2.reshape(1, OUT).astype(np.float16)
    ones3 = np.ones((3, QT), dtype=ml_dtypes.bfloat16)

    in_maps = []
    for c in range(NCORE):
        nrows = slice(c * NL, (c + 1) * NL)
        mrows = slice(c * ML, (c + 1) * ML)
        pc = pos[nrows]                       # [NL, 3]
        pk = pos_skip[mrows]                  # [ML, 3]
        # x side: 2*x splits, transposed per graph [GPC, 3, NP]
        x2T = (2.0 * pc).reshape(GPC, NP, 3).transpose(0, 2, 1)
        xh = np.ascontiguousarray(x2T.astype(ml_dtypes.bfloat16))
        xr = (x2T - xh.astype(np.float32))
        xm = np.ascontiguousarray(xr.astype(ml_dtypes.bfloat16))
        xl = np.ascontiguousarray((xr - xm.astype(np.float32)).astype(ml_dtypes.bfloat16))
        # negated ||x||^2, 3-way bf16 split, [GPC, 3, NP]
        n2 = (-np.sum(pc * pc, axis=1, dtype=np.float32)).reshape(GPC, 1, NP)
        n2h = n2.astype(ml_dtypes.bfloat16)
        n2r = n2 - n2h.astype(np.float32)
        n2m = n2r.astype(ml_dtypes.bfloat16)
        n2l = (n2r - n2m.astype(np.float32)).astype(ml_dtypes.bfloat16)
        x2n = np.ascontiguousarray(np.concatenate(
            [n2h, n2m, n2l], axis=1).astype(ml_dtypes.bfloat16))
        # y side: unscaled splits [GPC, 3, MP]
        yT = pk.reshape(GPC, MP, 3).transpose(0, 2, 1)
        yh = np.ascontiguousarray(yT.astype(ml_dtypes.bfloat16))
        yr = (yT - yh.astype(np.float32))
        ym = np.ascontiguousarray(yr.astype(ml_dtypes.bfloat16))
        yl = np.ascontiguousarray((yr - ym.astype(np.float32)).astype(ml_dtypes.bfloat16))
        in_maps.append(dict(
            x_in=np.ascontiguousarray(x[nrows]),
            lf_in=np.ascontiguousarray(lframes[nrows].reshape(NL, 9)),
            lfs_in=np.ascontiguousarray(lframes_skip[mrows].reshape(ML, 9)),
            pos_in=np.ascontiguousarray(pc),
            psk_in=np.ascontiguousarray(pk),
            xh_in=xh, xm_in=xm, xl_in=xl, x2n_in=x2n,
            yh_in=yh, ym_in=ym, yl_in=yl,
            xskT_in=np.ascontiguousarray(x_skip[mrows].T.astype(np.float16)),
            w1_in=W1p, b1_in=b1c, w2_in=W2h, b2_in=b2c,
            ones3_in=ones3,
        ))

    trace = bool(int(os.environ.get("KERNEL_TRACE", "0")))
    res = run_bass_kernel_spmd(nc, in_maps, core_ids=list(range(NCORE)),
                               trace=trace)
    _CACHE["last_exec_time_ns"] = res.exec_time_ns

    out = np.empty((B * MP, OUT), dtype=np.float32)
    for c in range(NCORE):
        out[c * ML:(c + 1) * ML, :] = np.asarray(res.results[c]["outT"]).T
    return (out, pos_skip, batch_skip, lframes_skip)


# revision 19
# speedup vs baseline: 1.1344x; 1.1344x over previous
"""Trainium2 Bass kernel for nn_FPModule (gnn message passing / FPModule).

Sharding: 2 graphs per core x 8 cores. Per core:
  - KNN scores u = 2*y.x - ||x||^2 on PE via 21-row 3-way-bf16-split matmul
    (error ~2e-7 rel, fp32-accurate ranking).
  - top-3 via DVE max8 + max_index (exact fp32).
  - interp weights from score values: d_k = ysq - v_k (clamped), w=1/d.
  - z = Rx^T x per coarse node (fp16) staged in DRAM; batched indirect-DMA
    gather of 12 rows/partition; interp on ACT (scale-AP), rotation by Ry via
    broadcast-AP DVE ops; fp16 MLP on PE; output written feature-major,
    host transposes back.
"""
import os
import numpy as np
from contextlib import ExitStack

import ml_dtypes
import concourse.bass as bass
import concourse.bacc as bacc
import concourse.mybir as mybir
from concourse.tile import TileContext
from concourse.bass import IndirectOffsetOnAxis
from concourse.masks import make_identity

DT = mybir.dt
AL = mybir.AluOpType
AF = mybir.ActivationFunctionType

NCORE = 8
B = 16
GPC = 2               # graphs per core
NP = 1024             # coarse per graph
MP = 4096             # queries per graph
NL = GPC * NP         # 2048 coarse rows per core
ML = GPC * MP         # 8192 query rows per core
NVEC = 64
FD = 192
FSK = 64
HID = 256
OUT = 128
QT = 128              # queries per tile
NT = MP // QT         # 32 tiles per graph
BT = 4                # tile batch

_CACHE = {}


def _build_nc():
    nc = bacc.Bacc("TRN2", target_bir_lowering=False, debug=False)
    dt = nc.dram_tensor
    x_in = dt("x_in", [NL, FD], DT.float32, kind="ExternalInput")
    lf_in = dt("lf_in", [NL, 9], DT.float32, kind="ExternalInput")
    lfs_in = dt("lfs_in", [ML, 9], DT.float32, kind="ExternalInput")
    pos_in = dt("pos_in", [NL, 3], DT.float32, kind="ExternalInput")
    psk_in = dt("psk_in", [ML, 3], DT.float32, kind="ExternalInput")
    xh_in = dt("xh_in", [GPC, 3, NP], DT.bfloat16, kind="ExternalInput")
    xm_in = dt("xm_in", [GPC, 3, NP], DT.bfloat16, kind="ExternalInput")
    xl_in = dt("xl_in", [GPC, 3, NP], DT.bfloat16, kind="ExternalInput")
    x2n_in = dt("x2n_in", [GPC, 3, NP], DT.bfloat16, kind="ExternalInput")
    yh_in = dt("yh_in", [GPC, 3, MP], DT.bfloat16, kind="ExternalInput")
    ym_in = dt("ym_in", [GPC, 3, MP], DT.bfloat16, kind="ExternalInput")
    yl_in = dt("yl_in", [GPC, 3, MP], DT.bfloat16, kind="ExternalInput")
    xskT_in = dt("xskT_in", [FSK, ML], DT.float16, kind="ExternalInput")
    w1_in = dt("w1_in", [FD + FSK, HID], DT.float16, kind="ExternalInput")
    b1_in = dt("b1_in", [HID, 1], DT.float32, kind="ExternalInput")
    w2_in = dt("w2_in", [HID, OUT], DT.float16, kind="ExternalInput")
    b2_in = dt("b2_in", [1, OUT], DT.float16, kind="ExternalInput")
    ones3_in = dt("ones3_in", [3, QT], DT.bfloat16, kind="ExternalInput")
    zs = [dt("z_%d" % g, [NP, FD], DT.float16, kind="Internal") for g in range(GPC)]
    outT = dt("outT", [OUT, ML], DT.float32, kind="ExternalOutput")

    with TileContext(nc) as tc, ExitStack() as ctx:
        const = ctx.enter_context(tc.tile_pool(name="const", bufs=1))
        xpool = ctx.enter_context(tc.tile_pool(name="xg", bufs=2))
        zpool = ctx.enter_context(tc.tile_pool(name="zp", bufs=3))
        ypool = ctx.enter_context(tc.tile_pool(name="yp", bufs=10))
        upool = ctx.enter_context(tc.tile_pool(name="up", bufs=2, space="PSUM"))
        usbp = ctx.enter_context(tc.tile_pool(name="usb", bufs=4))
        smpool = ctx.enter_context(tc.tile_pool(name="sm", bufs=4))
        gpool = ctx.enter_context(tc.tile_pool(name="gp", bufs=3))
        spool = ctx.enter_context(tc.tile_pool(name="sp", bufs=3))
        hpool = ctx.enter_context(tc.tile_pool(name="hp", bufs=3))
        mpsum = ctx.enter_context(tc.tile_pool(name="mp", bufs=1, space="PSUM"))
        tpsum = ctx.enter_context(tc.tile_pool(name="tp", bufs=2, space="PSUM"))

        # ---- constants ----
        w1t = const.tile([128, 2, HID], DT.float16)
        nc.sync.dma_start(out=w1t[:, 0, :], in_=w1_in[0:128, :])
        nc.sync.dma_start(out=w1t[:, 1, :], in_=w1_in[128:256, :])
        w2t = const.tile([128, 2, OUT], DT.float16)
        nc.sync.dma_start(out=w2t[:, 0, :], in_=w2_in[0:128, :])
        nc.sync.dma_start(out=w2t[:, 1, :], in_=w2_in[128:256, :])
        b1t = const.tile([128, 2], DT.float32)
        nc.sync.dma_start(out=b1t[:, 0:1], in_=b1_in[0:128, :])
        nc.sync.dma_start(out=b1t[:, 1:2], in_=b1_in[128:256, :])
        b2t = const.tile([1, OUT], DT.float16)
        nc.sync.dma_start(out=b2t[:], in_=b2_in[:])
        onesrow = const.tile([1, BT * QT], DT.float16)
        nc.vector.memset(onesrow[:], 1.0)
        ident = const.tile([128, 128], DT.float16)
        make_identity(nc, ident[:])

        for g in range(GPC):
            # ---- X moving tile [21, NP] ----
            Xg = xpool.tile([21, NP], DT.bfloat16)
            nc.sync.dma_start(out=Xg[0:3, :], in_=xh_in[g])
            nc.sync.dma_start(out=Xg[3:6, :], in_=xm_in[g])
            nc.sync.dma_start(out=Xg[6:9, :], in_=xh_in[g])
            nc.sync.dma_start(out=Xg[9:12, :], in_=xl_in[g])
            nc.sync.dma_start(out=Xg[12:15, :], in_=xh_in[g])
            nc.sync.dma_start(out=Xg[15:18, :], in_=xm_in[g])
            # x2 rows: negated ||x||^2 split into 3 bf16 (host-computed)
            nc.sync.dma_start(out=Xg[18:21, :], in_=x2n_in[g])

            # ---- ysq for this graph: [128, NT] ----
            yrm = zpool.tile([128, NT, 3], DT.float32)
            nc.sync.dma_start(
                out=yrm[:],
                in_=psk_in[g * MP:(g + 1) * MP, :].rearrange("(t p) d -> p t d", p=128))
            ysq = smpool.tile([128, NT], DT.float32, tag="ysq")
            sq2 = zpool.tile([128, NT, 3], DT.float32)
            nc.vector.tensor_tensor(sq2[:], yrm[:], yrm[:], op=AL.mult)
            nc.vector.tensor_reduce(ysq[:], sq2[:], axis=mybir.AxisListType.X, op=AL.add)

            # ---- z phase: z = Rx^T x, fp16, layout z[c, 3v+a] ----
            for t0 in range(0, NP // QT, BT):
                xt = zpool.tile([128, BT, FD], DT.float32, tag="zx")
                nc.sync.dma_start(
                    out=xt[:],
                    in_=x_in[g * NP + t0 * 128:g * NP + (t0 + BT) * 128, :]
                    .rearrange("(t p) f -> p t f", p=128))
                lft = zpool.tile([128, BT, 9], DT.float32, tag="zlf")
                nc.sync.dma_start(
                    out=lft[:],
                    in_=lf_in[g * NP + t0 * 128:g * NP + (t0 + BT) * 128, :]
                    .rearrange("(t p) f -> p t f", p=128))
                pb = []
                for b in range(3):
                    pbt = zpool.tile([128, BT, FD], DT.float16, tag="zpb%d" % b,
                                     name="pb%d" % b)
                    pb.append(pbt)
                for b in range(3):
                    for t in range(BT):
                        for a in range(3):
                            # p_b[:, t, 3v+a] = x[:, t, 3v+b] * Rx[b, a]
                            nc.scalar.activation(
                                pb[b][:, t, a:FD:3], xt[:, t, b:FD:3], AF.Copy,
                                bias=0.0, scale=lft[:, t, 3 * b + a:3 * b + a + 1])
                zt = zpool.tile([128, BT, FD], DT.float16, tag="zt")
                nc.vector.tensor_tensor(zt[:], pb[0][:], pb[1][:], op=AL.add)
                nc.vector.tensor_tensor(zt[:], zt[:], pb[2][:], op=AL.add)
                nc.sync.dma_start(
                    out=zs[g][t0 * 128:(t0 + BT) * 128, :]
                    .rearrange("(t p) f -> p t f", p=128),
                    in_=zt[:])

            # ---- query tiles ----
            for jb in range(0, NT, BT):
                vb = smpool.tile([128, BT, 8], DT.float32, tag="vb")
                ib = smpool.tile([128, BT, 8], DT.uint32, tag="ib")
                for jj in range(BT):
                    j = jb + jj
                    Y = ypool.tile([21, QT], DT.bfloat16)
                    c0 = j * QT
                    nc.sync.dma_start(out=Y[0:3, :], in_=yh_in[g, :, c0:c0 + QT])
                    nc.sync.dma_start(out=Y[3:6, :], in_=yh_in[g, :, c0:c0 + QT])
                    nc.sync.dma_start(out=Y[6:9, :], in_=ym_in[g, :, c0:c0 + QT])
                    nc.sync.dma_start(out=Y[9:12, :], in_=yh_in[g, :, c0:c0 + QT])
                    nc.sync.dma_start(out=Y[12:15, :], in_=yl_in[g, :, c0:c0 + QT])
                    nc.sync.dma_start(out=Y[15:18, :], in_=ym_in[g, :, c0:c0 + QT])
                    nc.sync.dma_start(out=Y[18:21, :], in_=ones3_in[:])
                    up = upool.tile([128, NP], DT.float32)
                    nc.tensor.matmul(up[:, 0:512], lhsT=Y[:], rhs=Xg[:, 0:512],
                                     start=True, stop=True)
                    nc.tensor.matmul(up[:, 512:1024], lhsT=Y[:], rhs=Xg[:, 512:1024],
                                     start=True, stop=True)
                    usb = usbp.tile([128, NP], DT.float32)
                    nc.scalar.activation(usb[:], up[:], AF.Copy, bias=0.0, scale=1.0)
                    nc.vector.max(vb[:, jj, :], usb[:])
                    nc.vector.max_index(ib[:, jj, :], vb[:, jj, :], usb[:])

                # batched small ops for BT tiles
                tneg = smpool.tile([128, BT, 3], DT.float32, tag="tneg")
                nc.vector.tensor_tensor(
                    tneg[:], vb[:, :, 0:3],
                    ysq[:, jb:jb + BT, None].to_broadcast([128, BT, 3]),
                    op=AL.subtract)
                nc.vector.tensor_scalar_min(tneg[:], tneg[:], -1e-16)
                wn = smpool.tile([128, BT, 3], DT.float32, tag="wn")
                with nc.allow_low_precision(reason="1/d weights"):
                    nc.vector.reciprocal(wn[:], tneg[:])
                swn = smpool.tile([128, BT], DT.float32, tag="swn")
                nc.vector.tensor_reduce(swn[:], wn[:], axis=mybir.AxisListType.X,
                                        op=AL.add)
                rsw = smpool.tile([128, BT], DT.float32, tag="rsw")
                with nc.allow_low_precision(reason="1/sum"):
                    nc.vector.reciprocal(rsw[:], swn[:])
                wt = smpool.tile([128, BT, 3], DT.float32, tag="wt")
                nc.vector.tensor_tensor(
                    wt[:], wn[:], rsw[:, :, None].to_broadcast([128, BT, 3]),
                    op=AL.mult)
                ig = smpool.tile([128, BT, 3], DT.uint32, tag="ig")
                nc.vector.tensor_scalar(ig[:], ib[:, :, 0:3], 0, None, op0=AL.add)

                # gather 12 z rows per partition
                gb = gpool.tile([128, BT * 3, FD], DT.float16)
                for ii in range(BT * 3):
                    nc.gpsimd.indirect_dma_start(
                        out=gb[:, ii, :], out_offset=None, in_=zs[g][:],
                        in_offset=IndirectOffsetOnAxis(
                            ap=ig[:, ii // 3, ii % 3:ii % 3 + 1], axis=0))

                # interp on ACT: s = sum_k w_k * g_k
                s16 = spool.tile([128, BT, FD], DT.float16, tag="s16")
                t16 = spool.tile([128, BT, FD], DT.float16, tag="t16")
                for jj in range(BT):
                    nc.scalar.activation(s16[:, jj, :], gb[:, 3 * jj + 0, :], AF.Copy,
                                         bias=0.0, scale=wt[:, jj, 0:1])
                    nc.scalar.activation(t16[:, jj, :], gb[:, 3 * jj + 1, :], AF.Copy,
                                         bias=0.0, scale=wt[:, jj, 1:2])
                nc.vector.tensor_tensor(s16[:], s16[:], t16[:], op=AL.add)
                for jj in range(BT):
                    nc.scalar.activation(t16[:, jj, :], gb[:, 3 * jj + 2, :], AF.Copy,
                                         bias=0.0, scale=wt[:, jj, 2:3])
                nc.vector.tensor_tensor(s16[:], s16[:], t16[:], op=AL.add)

                # rotation y[q, a*64+v] = sum_b Ry[3a+b] * s[q, 3v+b]  (DVE bcast)
                ry = smpool.tile([128, BT, 9], DT.float32, tag="ry")
                for jj in range(BT):
                    q0 = g * MP + (jb + jj) * QT
                    nc.sync.dma_start(
                        out=ry[:, jj, :],
                        in_=lfs_in[q0:q0 + QT, :])
                y16 = hpool.tile([128, BT, 3, NVEC], DT.float16, tag="y16")
                u16 = hpool.tile([128, BT, 3, NVEC], DT.float16, tag="u16")

                def ry_view(b):
                    return ry[:, :, b:9:3, None].to_broadcast([128, BT, 3, NVEC])

                def s_view(b):
                    return s16[:, :, None, b:FD:3].to_broadcast([128, BT, 3, NVEC])

                nc.vector.tensor_tensor(y16[:], ry_view(0), s_view(0), op=AL.mult)
                nc.vector.tensor_tensor(u16[:], ry_view(1), s_view(1), op=AL.mult)
                nc.vector.tensor_tensor(y16[:], y16[:], u16[:], op=AL.add)
                nc.vector.tensor_tensor(u16[:], ry_view(2), s_view(2), op=AL.mult)
                nc.vector.tensor_tensor(y16[:], y16[:], u16[:], op=AL.add)

                # transposes to feature-major h
                hA = hpool.tile([128, BT, QT], DT.float16, tag="hA")
                hB = hpool.tile([128, BT, QT], DT.float16, tag="hB")
                yflat = y16[:].rearrange("p t a v -> p t (a v)")
                for jj in range(BT):
                    tp1 = tpsum.tile([128, QT], DT.float16, tag="tp")
                    nc.tensor.transpose(tp1[:], yflat[:, jj, 0:128], ident[:])
                    nc.scalar.activation(hA[:, jj, :], tp1[:], AF.Copy,
                                         bias=0.0, scale=1.0)
                    tp2 = tpsum.tile([64, QT], DT.float16, tag="tp")
                    nc.tensor.transpose(tp2[:], yflat[:, jj, 128:192], ident[:])
                    nc.scalar.activation(hB[0:64, jj, :], tp2[:], AF.Copy,
                                         bias=0.0, scale=1.0)
                    q0 = g * MP + (jb + jj) * QT
                    nc.sync.dma_start(out=hB[64:128, jj, :],
                                      in_=xskT_in[:, q0:q0 + QT])

                # MLP
                ps1a = mpsum.tile([128, BT * QT], DT.float32, tag="ps1")
                ps1b = mpsum.tile([128, BT * QT], DT.float32, tag="ps1")
                hAf = hA[:].rearrange("p t q -> p (t q)")
                hBf = hB[:].rearrange("p t q -> p (t q)")
                nc.tensor.matmul(ps1a[:], lhsT=w1t[:, 0, 0:128], rhs=hAf,
                                 start=True, stop=False)
                nc.tensor.matmul(ps1a[:], lhsT=w1t[:, 1, 0:128], rhs=hBf,
                                 start=False, stop=True)
                nc.tensor.matmul(ps1b[:], lhsT=w1t[:, 0, 128:256], rhs=hAf,
                                 start=True, stop=False)
                nc.tensor.matmul(ps1b[:], lhsT=w1t[:, 1, 128:256], rhs=hBf,
                                 start=False, stop=True)
                h2a = hpool.tile([128, BT * QT], DT.float16, tag="h2a")
                h2b = hpool.tile([128, BT * QT], DT.float16, tag="h2b")
                nc.scalar.activation(h2a[:], ps1a[:], AF.Relu,
                                     bias=b1t[:, 0:1], scale=1.0)
                nc.scalar.activation(h2b[:], ps1b[:], AF.Relu,
                                     bias=b1t[:, 1:2], scale=1.0)
                ps2 = mpsum.tile([128, BT * QT], DT.float32, tag="ps2")
                nc.tensor.matmul(ps2[:], lhsT=w2t[:, 0, :], rhs=h2a[:],
                                 start=True, stop=False)
                nc.tensor.matmul(ps2[:], lhsT=w2t[:, 1, :], rhs=h2b[:],
                                 start=False, stop=False)
                nc.tensor.matmul(ps2[:], lhsT=b2t[:], rhs=onesrow[:],
                                 start=False, stop=True)
                osb = hpool.tile([128, BT * QT], DT.float32, tag="osb")
                nc.scalar.activation(osb[:], ps2[:], AF.Copy, bias=0.0, scale=1.0)
                col0 = g * MP + jb * QT
                nc.sync.dma_start(out=outT[:, col0:col0 + BT * QT], in_=osb[:])
    return nc


def _split3(a):
    """3-way bf16 split of fp32 array: a ~= h + m + l (each bf16)."""
    h = a.astype(ml_dtypes.bfloat16)
    r = (a - h.astype(np.float32)).astype(np.float32)
    m = r.astype(ml_dtypes.bfloat16)
    l = (r - m.astype(np.float32)).astype(ml_dtypes.bfloat16)
    return h, m, l


def kernel(x, pos, pos_skip, x_skip, lframes, lframes_skip,
           W1, b1, W2, b2, batch, batch_skip):
    from concourse.bass_utils import run_bass_kernel_spmd

    x = np.asarray(x, dtype=np.float32)
    pos = np.asarray(pos, dtype=np.float32)
    pos_skip = np.asarray(pos_skip, dtype=np.float32)
    x_skip = np.asarray(x_skip, dtype=np.float32)
    lframes = np.asarray(lframes, dtype=np.float32)
    lframes_skip = np.asarray(lframes_skip, dtype=np.float32)
    W1 = np.asarray(W1, dtype=np.float32)
    b1 = np.asarray(b1, dtype=np.float32)
    W2 = np.asarray(W2, dtype=np.float32)
    b2 = np.asarray(b2, dtype=np.float32)

    if "nc" not in _CACHE:
        nc_ = _build_nc()
        nc_.finalize()
        _CACHE["nc"] = nc_
    nc = _CACHE["nc"]

    # W1 row permutation: y features (3v+a) -> (a*64+v); x_skip rows unchanged
    perm = np.empty(FD + FSK, dtype=np.int64)
    for v in range(NVEC):
        for a in range(3):
            perm[a * NVEC + v] = 3 * v + a
    perm[FD:] = np.arange(FD, FD + FSK)
    W1p = W1[perm, :].astype(np.float16)
    W2h = W2.astype(np.float16)
    b1c = b1.reshape(HID, 1).astype(np.float32)
    b2c = b2.reshape(1, OUT).astype(np.float16)
    ones3 = np.ones((3, QT), dtype=ml_dtypes.bfloat16)

    in_maps = []
    for c in range(NCORE):
        nrows = slice(c * NL, (c + 1) * NL)
        mrows = slice(c * ML, (c + 1) * ML)
        pc = pos[nrows]                       # [NL, 3]
        pk = pos_skip[mrows]                  # [ML, 3]
        # x side: 2*x splits, transposed per graph [GPC, 3, NP]
        x2T = (2.0 * pc).reshape(GPC, NP, 3).transpose(0, 2, 1)
        xh = np.ascontiguousarray(x2T.astype(ml_dtypes.bfloat16))
        xr = (x2T - xh.astype(np.float32))
        xm = np.ascontiguousarray(xr.astype(ml_dtypes.bfloat16))
        xl = np.ascontiguousarray((xr - xm.astype(np.float32)).astype(ml_dtypes.bfloat16))
        # negated ||x||^2, 3-way bf16 split, [GPC, 3, NP]
        n2 = (-np.sum(pc * pc, axis=1, dtype=np.float32)).reshape(GPC, 1, NP)
        n2h = n2.astype(ml_dtypes.bfloat16)
        n2r = n2 - n2h.astype(np.float32)
        n2m = n2r.astype(ml_dtypes.bfloat16)
        n2l = (n2r - n2m.astype(np.float32)).astype(ml_dtypes.bfloat16)
        x2n = np.ascontiguousarray(np.concatenate(
            [n2h, n2m, n2l], axis=1).astype(ml_dtypes.bfloat16))
        # y side: unscaled splits [GPC, 3, MP]
        yT = pk.reshape(GPC, MP, 3).transpose(0, 2, 1)
        yh = np.ascontiguousarray(yT.astype(ml_dtypes.bfloat16))
        yr = (yT - yh.astype(np.float32))
        ym = np.ascontiguousarray(yr.astype(ml_dtypes.bfloat16))
        yl = np.ascontiguousarray((yr - ym.astype(np.float32)).astype(ml_dtypes.bfloat16))
        in_maps.append(dict(
            x_in=np.ascontiguousarray(x[nrows]),
            lf_in=np.ascontiguousarray(lframes[nrows].reshape(NL, 9)),
            lfs_in=np.ascontiguousarray(lframes_skip[mrows].reshape(ML, 9)),
            pos_in=np.ascontiguousarray(pc),
            psk_in=np.ascontiguousarray(pk),
            xh_in=xh, xm_in=xm, xl_in=xl, x2n_in=x2n,
            yh_in=yh, ym_in=ym, yl_in=yl,
            xskT_in=np.ascontiguousarray(x_skip[mrows].T.astype(np.float16)),
            w1_in=W1p, b1_in=b1c, w2_in=W2h, b2_in=b2c,
            ones3_in=ones3,
        ))

    trace = bool(int(os.environ.get("KERNEL_TRACE", "0")))
    res = run_bass_kernel_spmd(nc, in_maps, core_ids=list(range(NCORE)),
                               trace=trace)
    _CACHE["last_exec_time_ns"] = res.exec_time_ns

    out = np.empty((B * MP, OUT), dtype=np.float32)
    for c in range(NCORE):
        out[c * ML:(c + 1) * ML, :] = np.asarray(res.results[c]["outT"]).T
    return (out, pos_skip, batch_skip, lframes_skip)


# revision 20
# speedup vs baseline: 1.2654x; 1.1155x over previous
"""Trainium2 Bass kernel for nn_FPModule (gnn message passing / FPModule).

Sharding: 2 graphs per core x 8 cores. Per core:
  - KNN scores u = 2*y.x - ||x||^2 on PE via 21-row 3-way-bf16-split matmul
    (error ~2e-7 rel, fp32-accurate ranking).
  - top-3 via DVE max8 + max_index (exact fp32).
  - interp weights from score values: d_k = ysq - v_k (clamped), w=1/d.
  - z = Rx^T x per coarse node (fp16) staged in DRAM; batched indirect-DMA
    gather of 12 rows/partition; interp on ACT (scale-AP), rotation by Ry via
    broadcast-AP DVE ops; fp16 MLP on PE; output written feature-major,
    host transposes back.
"""
import os
import numpy as np
from contextlib import ExitStack

import ml_dtypes
import concourse.bass as bass
import concourse.bacc as bacc
import concourse.mybir as mybir
from concourse.tile import TileContext
from concourse.bass import IndirectOffsetOnAxis
from concourse.masks import make_identity

DT = mybir.dt
AL = mybir.AluOpType
AF = mybir.ActivationFunctionType

NCORE = 8
B = 16
GPC = 2               # graphs per core
NP = 1024             # coarse per graph
MP = 4096             # queries per graph
NL = GPC * NP         # 2048 coarse rows per core
ML = GPC * MP         # 8192 query rows per core
NVEC = 64
FD = 192
FSK = 64
HID = 256
OUT = 128
QT = 128              # queries per tile
NT = MP // QT         # 32 tiles per graph
BT = 4                # tile batch

_CACHE = {}


def _build_nc():
    nc = bacc.Bacc("TRN2", target_bir_lowering=False, debug=False)
    dt = nc.dram_tensor
    x_in = dt("x_in", [NL, FD], DT.float32, kind="ExternalInput")
    lf_in = dt("lf_in", [NL, 9], DT.float32, kind="ExternalInput")
    lfs_in = dt("lfs_in", [ML, 9], DT.float32, kind="ExternalInput")
    pos_in = dt("pos_in", [NL, 3], DT.float32, kind="ExternalInput")
    psk_in = dt("psk_in", [ML, 3], DT.float32, kind="ExternalInput")
    xh_in = dt("xh_in", [GPC, 3, NP], DT.bfloat16, kind="ExternalInput")
    xm_in = dt("xm_in", [GPC, 3, NP], DT.bfloat16, kind="ExternalInput")
    xl_in = dt("xl_in", [GPC, 3, NP], DT.bfloat16, kind="ExternalInput")
    x2n_in = dt("x2n_in", [GPC, 3, NP], DT.bfloat16, kind="ExternalInput")
    yh_in = dt("yh_in", [GPC, 3, MP], DT.bfloat16, kind="ExternalInput")
    ym_in = dt("ym_in", [GPC, 3, MP], DT.bfloat16, kind="ExternalInput")
    yl_in = dt("yl_in", [GPC, 3, MP], DT.bfloat16, kind="ExternalInput")
    xskT_in = dt("xskT_in", [FSK, ML], DT.float16, kind="ExternalInput")
    w1_in = dt("w1_in", [FD + FSK, HID], DT.float16, kind="ExternalInput")
    b1_in = dt("b1_in", [HID, 1], DT.float32, kind="ExternalInput")
    w2_in = dt("w2_in", [HID, OUT], DT.float16, kind="ExternalInput")
    b2_in = dt("b2_in", [1, OUT], DT.float16, kind="ExternalInput")
    ones3_in = dt("ones3_in", [3, QT], DT.bfloat16, kind="ExternalInput")
    zs = [dt("z_%d" % g, [NP, FD], DT.float16, kind="Internal") for g in range(GPC)]
    outT = dt("outT", [OUT, ML], DT.float32, kind="ExternalOutput")

    with TileContext(nc) as tc, ExitStack() as ctx:
        const = ctx.enter_context(tc.tile_pool(name="const", bufs=1))
        xpool = ctx.enter_context(tc.tile_pool(name="xg", bufs=2))
        zpool = ctx.enter_context(tc.tile_pool(name="zp", bufs=3))
        ypool = ctx.enter_context(tc.tile_pool(name="yp", bufs=10))
        upool = ctx.enter_context(tc.tile_pool(name="up", bufs=2, space="PSUM"))
        usbp = ctx.enter_context(tc.tile_pool(name="usb", bufs=4))
        smpool = ctx.enter_context(tc.tile_pool(name="sm", bufs=4))
        gpool = ctx.enter_context(tc.tile_pool(name="gp", bufs=3))
        spool = ctx.enter_context(tc.tile_pool(name="sp", bufs=3))
        hpool = ctx.enter_context(tc.tile_pool(name="hp", bufs=3))
        mpsum = ctx.enter_context(tc.tile_pool(name="mp", bufs=1, space="PSUM"))
        tpsum = ctx.enter_context(tc.tile_pool(name="tp", bufs=2, space="PSUM"))

        # ---- constants ----
        w1t = const.tile([128, 2, HID], DT.float16)
        nc.sync.dma_start(out=w1t[:, 0, :], in_=w1_in[0:128, :])
        nc.sync.dma_start(out=w1t[:, 1, :], in_=w1_in[128:256, :])
        w2t = const.tile([128, 2, OUT], DT.float16)
        nc.sync.dma_start(out=w2t[:, 0, :], in_=w2_in[0:128, :])
        nc.sync.dma_start(out=w2t[:, 1, :], in_=w2_in[128:256, :])
        b1t = const.tile([128, 2], DT.float32)
        nc.sync.dma_start(out=b1t[:, 0:1], in_=b1_in[0:128, :])
        nc.sync.dma_start(out=b1t[:, 1:2], in_=b1_in[128:256, :])
        b2t = const.tile([1, OUT], DT.float16)
        nc.sync.dma_start(out=b2t[:], in_=b2_in[:])
        onesrow = const.tile([1, BT * QT], DT.float16)
        nc.vector.memset(onesrow[:], 1.0)
        ident = const.tile([128, 128], DT.float16)
        make_identity(nc, ident[:])

        Xgs, ysqs = [], []
        for g in range(GPC):
            # ---- X moving tile [21, NP] ----
            Xg = xpool.tile([21, NP], DT.bfloat16)
            nc.sync.dma_start(out=Xg[0:3, :], in_=xh_in[g])
            nc.sync.dma_start(out=Xg[3:6, :], in_=xm_in[g])
            nc.sync.dma_start(out=Xg[6:9, :], in_=xh_in[g])
            nc.sync.dma_start(out=Xg[9:12, :], in_=xl_in[g])
            nc.sync.dma_start(out=Xg[12:15, :], in_=xh_in[g])
            nc.sync.dma_start(out=Xg[15:18, :], in_=xm_in[g])
            # x2 rows: negated ||x||^2 split into 3 bf16 (host-computed)
            nc.sync.dma_start(out=Xg[18:21, :], in_=x2n_in[g])
            Xgs.append(Xg)

            # ---- ysq for this graph: [128, NT] ----
            yrm = zpool.tile([128, NT, 3], DT.float32)
            nc.sync.dma_start(
                out=yrm[:],
                in_=psk_in[g * MP:(g + 1) * MP, :].rearrange("(t p) d -> p t d", p=128))
            ysq = smpool.tile([128, NT], DT.float32, tag="ysq")
            sq2 = zpool.tile([128, NT, 3], DT.float32)
            nc.vector.tensor_tensor(sq2[:], yrm[:], yrm[:], op=AL.mult)
            nc.vector.tensor_reduce(ysq[:], sq2[:], axis=mybir.AxisListType.X, op=AL.add)
            ysqs.append(ysq)

            # ---- z phase: z = Rx^T x, fp16, layout z[c, 3v+a] ----
            for t0 in range(0, NP // QT, BT):
                xt = zpool.tile([128, BT, FD], DT.float32, tag="zx")
                nc.sync.dma_start(
                    out=xt[:],
                    in_=x_in[g * NP + t0 * 128:g * NP + (t0 + BT) * 128, :]
                    .rearrange("(t p) f -> p t f", p=128))
                lft = zpool.tile([128, BT, 9], DT.float32, tag="zlf")
                nc.sync.dma_start(
                    out=lft[:],
                    in_=lf_in[g * NP + t0 * 128:g * NP + (t0 + BT) * 128, :]
                    .rearrange("(t p) f -> p t f", p=128))
                pb = []
                for b in range(3):
                    pbt = zpool.tile([128, BT, FD], DT.float16, tag="zpb%d" % b,
                                     name="pb%d" % b)
                    pb.append(pbt)
                for b in range(3):
                    for t in range(BT):
                        for a in range(3):
                            # p_b[:, t, 3v+a] = x[:, t, 3v+b] * Rx[b, a]
                            nc.scalar.activation(
                                pb[b][:, t, a:FD:3], xt[:, t, b:FD:3], AF.Copy,
                                bias=0.0, scale=lft[:, t, 3 * b + a:3 * b + a + 1])
                zt = zpool.tile([128, BT, FD], DT.float16, tag="zt")
                nc.vector.tensor_tensor(zt[:], pb[0][:], pb[1][:], op=AL.add)
                nc.vector.tensor_tensor(zt[:], zt[:], pb[2][:], op=AL.add)
                nc.sync.dma_start(
                    out=zs[g][t0 * 128:(t0 + BT) * 128, :]
                    .rearrange("(t p) f -> p t f", p=128),
                    in_=zt[:])


        for g in range(GPC):
            Xg = Xgs[g]
            ysq = ysqs[g]
            # ---- query tiles ----
            for jb in range(0, NT, BT):
                vb = smpool.tile([128, BT, 8], DT.float32, tag="vb")
                ib = smpool.tile([128, BT, 8], DT.uint32, tag="ib")
                for jj in range(BT):
                    j = jb + jj
                    Y = ypool.tile([21, QT], DT.bfloat16)
                    c0 = j * QT
                    nc.sync.dma_start(out=Y[0:3, :], in_=yh_in[g, :, c0:c0 + QT])
                    nc.sync.dma_start(out=Y[3:6, :], in_=yh_in[g, :, c0:c0 + QT])
                    nc.sync.dma_start(out=Y[6:9, :], in_=ym_in[g, :, c0:c0 + QT])
                    nc.sync.dma_start(out=Y[9:12, :], in_=yh_in[g, :, c0:c0 + QT])
                    nc.sync.dma_start(out=Y[12:15, :], in_=yl_in[g, :, c0:c0 + QT])
                    nc.sync.dma_start(out=Y[15:18, :], in_=ym_in[g, :, c0:c0 + QT])
                    nc.sync.dma_start(out=Y[18:21, :], in_=ones3_in[:])
                    up = upool.tile([128, NP], DT.float32)
                    nc.tensor.matmul(up[:, 0:512], lhsT=Y[:], rhs=Xg[:, 0:512],
                                     start=True, stop=True)
                    nc.tensor.matmul(up[:, 512:1024], lhsT=Y[:], rhs=Xg[:, 512:1024],
                                     start=True, stop=True)
                    usb = usbp.tile([128, NP], DT.float32)
                    nc.scalar.activation(usb[:], up[:], AF.Copy, bias=0.0, scale=1.0)
                    nc.vector.max(vb[:, jj, :], usb[:])
                    nc.vector.max_index(ib[:, jj, :], vb[:, jj, :], usb[:])

                # batched small ops for BT tiles
                tneg = smpool.tile([128, BT, 3], DT.float32, tag="tneg")
                nc.vector.tensor_tensor(
                    tneg[:], vb[:, :, 0:3],
                    ysq[:, jb:jb + BT, None].to_broadcast([128, BT, 3]),
                    op=AL.subtract)
                nc.vector.tensor_scalar_min(tneg[:], tneg[:], -1e-16)
                wn = smpool.tile([128, BT, 3], DT.float32, tag="wn")
                with nc.allow_low_precision(reason="1/d weights"):
                    nc.vector.reciprocal(wn[:], tneg[:])
                swn = smpool.tile([128, BT], DT.float32, tag="swn")
                nc.vector.tensor_reduce(swn[:], wn[:], axis=mybir.AxisListType.X,
                                        op=AL.add)
                rsw = smpool.tile([128, BT], DT.float32, tag="rsw")
                with nc.allow_low_precision(reason="1/sum"):
                    nc.vector.reciprocal(rsw[:], swn[:])
                wt = smpool.tile([128, BT, 3], DT.float32, tag="wt")
                nc.vector.tensor_tensor(
                    wt[:], wn[:], rsw[:, :, None].to_broadcast([128, BT, 3]),
                    op=AL.mult)
                ig = smpool.tile([128, BT, 3], DT.uint32, tag="ig")
                nc.vector.tensor_scalar(ig[:], ib[:, :, 0:3], 0, None, op0=AL.add)

                # gather 12 z rows per partition
                gb = gpool.tile([128, BT * 3, FD], DT.float16)
                for ii in range(BT * 3):
                    nc.gpsimd.indirect_dma_start(
                        out=gb[:, ii, :], out_offset=None, in_=zs[g][:],
                        in_offset=IndirectOffsetOnAxis(
                            ap=ig[:, ii // 3, ii % 3:ii % 3 + 1], axis=0))

                # interp on ACT: s = sum_k w_k * g_k
                s16 = spool.tile([128, BT, FD], DT.float16, tag="s16")
                t16 = spool.tile([128, BT, FD], DT.float16, tag="t16")
                for jj in range(BT):
                    nc.scalar.activation(s16[:, jj, :], gb[:, 3 * jj + 0, :], AF.Copy,
                                         bias=0.0, scale=wt[:, jj, 0:1])
                    nc.scalar.activation(t16[:, jj, :], gb[:, 3 * jj + 1, :], AF.Copy,
                                         bias=0.0, scale=wt[:, jj, 1:2])
                nc.vector.tensor_tensor(s16[:], s16[:], t16[:], op=AL.add)
                for jj in range(BT):
                    nc.scalar.activation(t16[:, jj, :], gb[:, 3 * jj + 2, :], AF.Copy,
                                         bias=0.0, scale=wt[:, jj, 2:3])
                nc.vector.tensor_tensor(s16[:], s16[:], t16[:], op=AL.add)

                # rotation y[q, a*64+v] = sum_b Ry[3a+b] * s[q, 3v+b]  (DVE bcast)
                ry = smpool.tile([128, BT, 9], DT.float32, tag="ry")
                for jj in range(BT):
                    q0 = g * MP + (jb + jj) * QT
                    nc.sync.dma_start(
                        out=ry[:, jj, :],
                        in_=lfs_in[q0:q0 + QT, :])
                y16 = hpool.tile([128, BT, 3, NVEC], DT.float16, tag="y16")
                u16 = hpool.tile([128, BT, 3, NVEC], DT.float16, tag="u16")

                def ry_view(b):
                    return ry[:, :, b:9:3, None].to_broadcast([128, BT, 3, NVEC])

                def s_view(b):
                    return s16[:, :, None, b:FD:3].to_broadcast([128, BT, 3, NVEC])

                nc.vector.tensor_tensor(y16[:], ry_view(0), s_view(0), op=AL.mult)
                nc.vector.tensor_tensor(u16[:], ry_view(1), s_view(1), op=AL.mult)
                nc.vector.tensor_tensor(y16[:], y16[:], u16[:], op=AL.add)
                nc.vector.tensor_tensor(u16[:], ry_view(2), s_view(2), op=AL.mult)
                nc.vector.tensor_tensor(y16[:], y16[:], u16[:], op=AL.add)

                # transposes to feature-major h
                hA = hpool.tile([128, BT, QT], DT.float16, tag="hA")
                hB = hpool.tile([128, BT, QT], DT.float16, tag="hB")
                yflat = y16[:].rearrange("p t a v -> p t (a v)")
                for jj in range(BT):
                    tp1 = tpsum.tile([128, QT], DT.float16, tag="tp")
                    nc.tensor.transpose(tp1[:], yflat[:, jj, 0:128], ident[:])
                    nc.scalar.activation(hA[:, jj, :], tp1[:], AF.Copy,
                                         bias=0.0, scale=1.0)
                    tp2 = tpsum.tile([64, QT], DT.float16, tag="tp")
                    nc.tensor.transpose(tp2[:], yflat[:, jj, 128:192], ident[:])
                    nc.scalar.activation(hB[0:64, jj, :], tp2[:], AF.Copy,
                                         bias=0.0, scale=1.0)
                    q0 = g * MP + (jb + jj) * QT
                    nc.sync.dma_start(out=hB[64:128, jj, :],
                                      in_=xskT_in[:, q0:q0 + QT])

                # MLP
                ps1a = mpsum.tile([128, BT * QT], DT.float32, tag="ps1")
                ps1b = mpsum.tile([128, BT * QT], DT.float32, tag="ps1")
                hAf = hA[:].rearrange("p t q -> p (t q)")
                hBf = hB[:].rearrange("p t q -> p (t q)")
                nc.tensor.matmul(ps1a[:], lhsT=w1t[:, 0, 0:128], rhs=hAf,
                                 start=True, stop=False)
                nc.tensor.matmul(ps1a[:], lhsT=w1t[:, 1, 0:128], rhs=hBf,
                                 start=False, stop=True)
                nc.tensor.matmul(ps1b[:], lhsT=w1t[:, 0, 128:256], rhs=hAf,
                                 start=True, stop=False)
                nc.tensor.matmul(ps1b[:], lhsT=w1t[:, 1, 128:256], rhs=hBf,
                                 start=False, stop=True)
                h2a = hpool.tile([128, BT * QT], DT.float16, tag="h2a")
                h2b = hpool.tile([128, BT * QT], DT.float16, tag="h2b")
                nc.scalar.activation(h2a[:], ps1a[:], AF.Relu,
                                     bias=b1t[:, 0:1], scale=1.0)
                nc.scalar.activation(h2b[:], ps1b[:], AF.Relu,
                                     bias=b1t[:, 1:2], scale=1.0)
                ps2 = mpsum.tile([128, BT * QT], DT.float32, tag="ps2")
                nc.tensor.matmul(ps2[:], lhsT=w2t[:, 0, :], rhs=h2a[:],
                                 start=True, stop=False)
                nc.tensor.matmul(ps2[:], lhsT=w2t[:, 1, :], rhs=h2b[:],
                                 start=False, stop=False)
                nc.tensor.matmul(ps2[:], lhsT=b2t[:], rhs=onesrow[:],
                                 start=False, stop=True)
                osb = hpool.tile([128, BT * QT], DT.float32, tag="osb")
                nc.scalar.activation(osb[:], ps2[:], AF.Copy, bias=0.0, scale=1.0)
                col0 = g * MP + jb * QT
                nc.sync.dma_start(out=outT[:, col0:col0 + BT * QT], in_=osb[:])
    return nc


def _split3(a):
    """3-way bf16 split of fp32 array: a ~= h + m + l (each bf16)."""
    h = a.astype(ml_dtypes.bfloat16)
    r = (a - h.astype(np.float32)).astype(np.float32)
    m = r.astype(ml_dtypes.bfloat16)
    l = (r - m.astype(np.float32)).astype(ml_dtypes.bfloat16)
    return h, m, l


def kernel(x, pos, pos_skip, x_skip, lframes, lframes_skip,
           W1, b1, W2, b2, batch, batch_skip):
    from concourse.bass_utils import run_bass_kernel_spmd

    x = np.asarray(x, dtype=np.float32)
    pos = np.asarray(pos, dtype=np.float32)
    pos_skip = np.asarray(pos_skip, dtype=np.float32)
    x_skip = np.asarray(x_skip, dtype=np.float32)
    lframes = np.asarray(lframes, dtype=np.float32)
    lframes_skip = np.asarray(lframes_skip, dtype=np.float32)
    W1 = np.asarray(W1, dtype=np.float32)
    b1 = np.asarray(b1, dtype=np.float32)
    W2 = np.asarray(W2, dtype=np.float32)
    b2 = np.asarray(b2, dtype=np.float32)

    if "nc" not in _CACHE:
        nc_ = _build_nc()
        nc_.finalize()
        _CACHE["nc"] = nc_
    nc = _CACHE["nc"]

    # W1 row permutation: y features (3v+a) -> (a*64+v); x_skip rows unchanged
    perm = np.empty(FD + FSK, dtype=np.int64)
    for v in range(NVEC):
        for a in range(3):
            perm[a * NVEC + v] = 3 * v + a
    perm[FD:] = np.arange(FD, FD + FSK)
    W1p = W1[perm, :].astype(np.float16)
    W2h = W2.astype(np.float16)
    b1c = b1.reshape(HID, 1).astype(np.float32)
    b2c = b2.reshape(1, OUT).astype(np.float16)
    ones3 = np.ones((3, QT), dtype=ml_dtypes.bfloat16)

    in_maps = []
    for c in range(NCORE):
        nrows = slice(c * NL, (c + 1) * NL)
        mrows = slice(c * ML, (c + 1) * ML)
        pc = pos[nrows]                       # [NL, 3]
        pk = pos_skip[mrows]                  # [ML, 3]
        # x side: 2*x splits, transposed per graph [GPC, 3, NP]
        x2T = (2.0 * pc).reshape(GPC, NP, 3).transpose(0, 2, 1)
        xh = np.ascontiguousarray(x2T.astype(ml_dtypes.bfloat16))
        xr = (x2T - xh.astype(np.float32))
        xm = np.ascontiguousarray(xr.astype(ml_dtypes.bfloat16))
        xl = np.ascontiguousarray((xr - xm.astype(np.float32)).astype(ml_dtypes.bfloat16))
        # negated ||x||^2, 3-way bf16 split, [GPC, 3, NP]
        n2 = (-np.sum(pc * pc, axis=1, dtype=np.float32)).reshape(GPC, 1, NP)
        n2h = n2.astype(ml_dtypes.bfloat16)
        n2r = n2 - n2h.astype(np.float32)
        n2m = n2r.astype(ml_dtypes.bfloat16)
        n2l = (n2r - n2m.astype(np.float32)).astype(ml_dtypes.bfloat16)
        x2n = np.ascontiguousarray(np.concatenate(
            [n2h, n2m, n2l], axis=1).astype(ml_dtypes.bfloat16))
        # y side: unscaled splits [GPC, 3, MP]
        yT = pk.reshape(GPC, MP, 3).transpose(0, 2, 1)
        yh = np.ascontiguousarray(yT.astype(ml_dtypes.bfloat16))
        yr = (yT - yh.astype(np.float32))
        ym = np.ascontiguousarray(yr.astype(ml_dtypes.bfloat16))
        yl = np.ascontiguousarray((yr - ym.astype(np.float32)).astype(ml_dtypes.bfloat16))
        in_maps.append(dict(
            x_in=np.ascontiguousarray(x[nrows]),
            lf_in=np.ascontiguousarray(lframes[nrows].reshape(NL, 9)),
            lfs_in=np.ascontiguousarray(lframes_skip[mrows].reshape(ML, 9)),
            pos_in=np.ascontiguousarray(pc),
            psk_in=np.ascontiguousarray(pk),
            xh_in=xh, xm_in=xm, xl_in=xl, x2n_in=x2n,
            yh_in=yh, ym_in=ym, yl_in=yl,
            xskT_in=np.ascontiguousarray(x_skip[mrows].T.astype(np.float16)),
            w1_in=W1p, b1_in=b1c, w2_in=W2h, b2_in=b2c,
            ones3_in=ones3,
        ))

    trace = bool(int(os.environ.get("KERNEL_TRACE", "0")))
    res = run_bass_kernel_spmd(nc, in_maps, core_ids=list(range(NCORE)),
                               trace=trace)
    _CACHE["last_exec_time_ns"] = res.exec_time_ns

    out = np.empty((B * MP, OUT), dtype=np.float32)
    for c in range(NCORE):
        out[c * ML:(c + 1) * ML, :] = np.asarray(res.results[c]["outT"]).T
    return (out, pos_skip, batch_skip, lframes_skip)
